# revision 30
# baseline (speedup 1.0000x reference)
"""Trainium2 Bass kernel for nn_DarcyResidual (P=256, B=128, 8 NeuronCores).

Math (reference):
    a = (x0 + 1.5) / 0.2,  p = (x1 + 0.9) / 115
    residual = -a*(p_d00 + p_d11) - a_d0*p_d0 - a_d1*p_d1 - 1
2nd-order central differences inside, 2nd-order one-sided at borders,
h = 1/256 on both axes.

Folded form (G = 5*65536/460):
    residual = (X0+1.5)*U4' + S1*R1' + C1a*C1p' - 1
with the host pre-scaling channel 1 by -G (so every X1-linear factor
carries the -G exactly once) and pre-adding 1.5 to channel 0:
    U4' = 4*(rowD2raw + colD2raw)(X1')   R1' = rowD1raw(X1')
    S1  = rowD1raw(X0')   C1*' = colD1raw shifts (host constants cancel
    in all derivative terms; one-sided edge cols use the same scaled xe).

All-bf16 pipeline: input is a single bf16 tensor (4.2MB/core), all row
stencils are bf16 matmuls (full PE rate; odd-element rhs offsets are
legal so the col-neighbor 4I shift matmuls read the padded X1 tile
directly).  PSUM tiles are [128,1024] (2 banks) so ScalarE evacuates
each stencil with one wide ACTIVATE.  DVE does the column stencils and
the three products at 2x bf16 with unshifted frames.  Output is bf16
(host upcasts); border columns j=0,255 come from the edge pipeline via
4 tiny SWDGE DMAs that never overlap the interior stores.
"""

import numpy as np

P = 256
B = 128
NCORES = 8
BPC = B // NCORES          # images per core = 16
CHUNKS = 4
BCH = BPC // CHUNKS        # images per chunk = 4
FCH = 2 * BCH * P          # chunk free size = 2048
GAMMA = 5.0 * 65536.0 / 460.0

_cache = {}


def _mats():
    D1 = np.zeros((P, P), dtype=np.float64)
    for i in range(1, P - 1):
        D1[i, i - 1] = -1.0
        D1[i, i + 1] = 1.0
    D1[0, 0:3] = [-3.0, 4.0, -1.0]
    D1[P - 1, P - 3:P] = [1.0, -4.0, 3.0]

    D2 = np.zeros((P, P), dtype=np.float64)
    for i in range(1, P - 1):
        D2[i, i - 1] = 1.0
        D2[i, i] = -2.0
        D2[i, i + 1] = 1.0
    D2[0, 0:4] = [2.0, -5.0, 4.0, -1.0]
    D2[P - 1, P - 4:P] = [-1.0, 4.0, -5.0, 2.0]
    return D1, D2


def _weights_main():
    """bf16 [128, 9, 128]: 0-3 D1 blocks, 4-7 4*(D2-2I) blocks, 8: 4I."""
    import ml_dtypes
    D1, D2 = _mats()
    WR2 = 4.0 * (D2 - 2.0 * np.eye(P))
    wtb = np.zeros((128, 9, 128), dtype=np.float64)
    for m in range(2):
        for kb in range(2):
            blk = lambda W: W[m * 128:(m + 1) * 128, kb * 128:(kb + 1) * 128].T
            wtb[:, m * 2 + kb, :] = blk(D1)
            wtb[:, 4 + m * 2 + kb, :] = blk(WR2)
    wtb[:, 8, :] = 4.0 * np.eye(128)
    return wtb.astype(ml_dtypes.bfloat16)


def _weights_edge():
    """f32 [128, 8, 128]: 0-3 D1 blocks, 4-7 4*D2 blocks (edge pipeline)."""
    D1, D2 = _mats()
    WR2E = 4.0 * D2
    wte = np.zeros((128, 8, 128), dtype=np.float32)
    for m in range(2):
        for kb in range(2):
            blk = lambda W: W[m * 128:(m + 1) * 128, kb * 128:(kb + 1) * 128].T
            wte[:, m * 2 + kb, :] = blk(D1)
            wte[:, 4 + m * 2 + kb, :] = blk(WR2E)
    return wte


def _build_program():
    from concourse import bacc
    import concourse.mybir as mybir
    from concourse.tile import TileContext

    f32 = mybir.dt.float32
    f32r = mybir.dt.float32r
    bf16 = mybir.dt.bfloat16
    ADD = mybir.AluOpType.add
    SUB = mybir.AluOpType.subtract
    MUL = mybir.AluOpType.mult
    COPY = mybir.ActivationFunctionType.Copy

    nc = bacc.Bacc("TRN2", target_bir_lowering=False, debug=False,
                   num_devices=NCORES)
    xb = nc.dram_tensor("xb", [128, 2, 2, BPC, P], bf16, kind="ExternalInput")
    xe = nc.dram_tensor("xe", [128, 2, 2, BPC, 8], f32r, kind="ExternalInput")
    wtbd = nc.dram_tensor("wtbd", [128, 9, 128], bf16, kind="ExternalInput")
    wted = nc.dram_tensor("wted", [128, 8, 128], f32r, kind="ExternalInput")
    yout = nc.dram_tensor("yout", [128, 2, BPC, P], bf16, kind="ExternalOutput")

    with TileContext(nc) as tc:
        with (
            tc.tile_pool(name="const", bufs=1) as cpool,
            tc.tile_pool(name="edge", bufs=1) as epool,
            tc.tile_pool(name="work", bufs=2) as pool,
            tc.tile_pool(name="psum", bufs=1, space="PSUM") as pp,
        ):
            # ---- chunk-0 input first (ch1 gates the first matmuls);
            # split so the first image-pair lands early ----
            X1t0 = pool.tile([128, FCH + 2], bf16, tag="x1", bufs=4)
            nc.sync.dma_start(
                out=X1t0[:, 1:FCH + 1].rearrange(
                    "p (k b j) -> p k b j", k=2, b=BCH)[:, :, 0:2, :],
                in_=xb[:, 1, :, 0:2, :])
            wtb = cpool.tile([128, 9, 128], bf16)
            nc.sync.dma_start(out=wtb[:], in_=wtbd[:])
            wte = cpool.tile([128, 8, 128], f32r)
            nc.sync.dma_start(out=wte[:], in_=wted[:])
            X0e = epool.tile([128, 2, BPC, 8], f32r)
            X1e = epool.tile([128, 2, BPC, 8], f32r)
            nc.sync.dma_start(out=X1e[:], in_=xe[:, 1])
            nc.sync.dma_start(out=X0e[:], in_=xe[:, 0])
            X0t0 = pool.tile([128, 2, BCH, P], bf16, tag="x0", bufs=4)
            nc.sync.dma_start(out=X0t0[:, :, 0:2, :],
                              in_=xb[:, 0, :, 0:2, :])
            nc.sync.dma_start(
                out=X1t0[:, 1:FCH + 1].rearrange(
                    "p (k b j) -> p k b j", k=2, b=BCH)[:, :, 2:BCH, :],
                in_=xb[:, 1, :, 2:BCH, :])
            nc.sync.dma_start(out=X0t0[:, :, 2:BCH, :],
                              in_=xb[:, 0, :, 2:BCH, :])

            def Wb(i):
                return wtb[:, i, :]

            def We(i):
                return wte[:, i, :]

            stt = nc.vector.scalar_tensor_tensor

            # ------------- edge pipeline (output cols j=0 and j=255) -------
            X0ef = X0e.rearrange("p k b c -> p (k b c)")
            X1ef = X1e.rearrange("p k b c -> p (k b c)")
            E1 = X1e.bitcast(f32).rearrange("p k b c -> p (k b) c")
            E0 = X0e.bitcast(f32).rearrange("p k b c -> p (k b) c")

            def et(name, d=2):
                return epool.tile([128, 2 * BPC, d], f32, name=name, tag=name)

            # edge psum, 2 banks: R2e in bank0 [0:256), R1e in bank1
            # [512:768); S1e reuses bank0 AFTER U4e consumes R2e (groups in
            # a bank must be sequential, never interleaved)
            pe_t = pp.tile([128, 1024], f32, name="edgep", tag="edgep")
            R2ef = pe_t[:, 0:256]
            R1ef = pe_t[:, 512:768]
            S1ef = pe_t[:, 0:256]
            for m in range(2):
                osl = slice(m * 128, (m + 1) * 128)
                for kb in range(2):
                    isl = slice(kb * 128, (kb + 1) * 128)
                    st, sp = kb == 0, kb == 1
                    nc.tensor.matmul(R1ef[:, osl], We(m * 2 + kb),
                                     X1ef[:, isl], start=st, stop=sp)
                    nc.tensor.matmul(R2ef[:, osl], We(4 + m * 2 + kb),
                                     X1ef[:, isl], start=st, stop=sp)

            # paired forward/mirrored one-sided diffs (half 0: j=0, half 1:
            # j=255 side)
            # SBUF-only edge elementwise runs on the (otherwise idle) Pool
            # engine; only PSUM-reading ops stay on DVE
            a1, b1, c1 = et("a1"), et("b1"), et("c1")
            a0, b0 = et("a0"), et("b0")
            nc.gpsimd.tensor_sub(a1[:], E1[:, :, 1:8:6], E1[:, :, 0:7:6])
            nc.gpsimd.tensor_sub(b1[:], E1[:, :, 2:7:4], E1[:, :, 1:6:4])
            nc.gpsimd.tensor_sub(c1[:], E1[:, :, 3:6:2], E1[:, :, 2:5:2])
            nc.gpsimd.tensor_sub(a0[:], E0[:, :, 1:8:6], E0[:, :, 0:7:6])
            nc.gpsimd.tensor_sub(b0[:], E0[:, :, 2:7:4], E0[:, :, 1:6:4])

            q, Z = et("q"), et("Z")
            C1pe, C1ae = et("C1pe"), et("C1ae")
            stt(q[:], b1[:], 3.0, c1[:], MUL, SUB)      # 3b - c
            stt(Z[:], a1[:], -2.0, q[:], MUL, ADD)      # -2a + 3b - c
            stt(C1pe[:], a1[:], 3.0, b1[:], MUL, SUB)   # 3a - b
            stt(C1ae[:], a0[:], 3.0, b0[:], MUL, SUB)

            RP2 = pe_t[:, 0:256].rearrange("p (x c) -> p x c", c=8)
            RP1 = pe_t[:, 512:768].rearrange("p (x c) -> p x c", c=8)
            U4e, tme, t2e = et("U4e"), et("tme"), et("t2e")
            stt(U4e[:, :, 0:1], Z[:, :, 0:1], 4.0, RP2[:, :, 0:1], MUL, ADD)
            stt(U4e[:, :, 1:2], Z[:, :, 1:2], -4.0, RP2[:, :, 7:8], MUL, ADD)

            # S1e into bank0 now that U4e has consumed R2e
            for m in range(2):
                osl = slice(m * 128, (m + 1) * 128)
                for kb in range(2):
                    isl = slice(kb * 128, (kb + 1) * 128)
                    nc.tensor.matmul(S1ef[:, osl], We(m * 2 + kb),
                                     X0ef[:, isl], start=kb == 0,
                                     stop=kb == 1)

            Scpe = epool.tile([128, 256], f32)
            nc.scalar.copy(out=Scpe[:], in_=S1ef[:])
            SP = Scpe.rearrange("p (x c) -> p x c", c=8)

            # E0 already holds x0+1.5 (host), E1 holds -G*x1
            nc.gpsimd.tensor_mul(tme[:], E0[:, :, 0:8:7], U4e[:])
            nc.vector.tensor_mul(t2e[:], SP[:, :, 0:8:7], RP1[:, :, 0:8:7])
            nc.gpsimd.tensor_add(tme[:], tme[:], t2e[:])
            nc.gpsimd.tensor_mul(C1ae[:], C1ae[:], C1pe[:])
            nc.gpsimd.tensor_add(tme[:], tme[:], C1ae[:])
            rese = epool.tile([128, 2, BPC, 2], bf16)
            nc.scalar.activation(
                rese.rearrange("p k b e -> p (k b) e"), tme[:], COPY,
                bias=-1.0, scale=1.0)
            # border columns: 4 tiny strided DMAs, disjoint from interior
            for k in range(2):
                nc.gpsimd.dma_start(out=yout[:, k, :, 0:1],
                                    in_=rese[:, k, :, 0:1])
                nc.gpsimd.dma_start(out=yout[:, k, :, P - 1:P],
                                    in_=rese[:, k, :, 1:2])

            # ------------- main pipeline, 4 chunks of 4 images -------------
            # res finalization + store lag 2 chunks behind so the Scalar
            # res-ACT never head-of-line blocks younger chunks' evacs
            pending = []

            def flush(ent):
                v3p, resp, b0p = ent
                nc.scalar.activation(
                    resp.rearrange("p k b j -> p (k b j)"), v3p[:],
                    COPY, bias=-1.0, scale=1.0)
                for k in range(2):
                    nc.gpsimd.dma_start(
                        out=yout[:, k, b0p:b0p + BCH, 1:P - 1],
                        in_=resp[:, k, :, 1:P - 1])

            for c in range(CHUNKS):
                b0c = c * BCH
                if len(pending) >= 2:
                    flush(pending.pop(0))
                if c == 0:
                    X1t, X0t = X1t0, X0t0
                else:
                    X1t = pool.tile([128, FCH + 2], bf16, tag="x1", bufs=4)
                    nc.sync.dma_start(
                        out=X1t[:, 1:FCH + 1].rearrange(
                            "p (k b j) -> p k b j", k=2, b=BCH),
                        in_=xb[:, 1, :, b0c:b0c + BCH, :])
                    X0t = pool.tile([128, 2, BCH, P], bf16, tag="x0", bufs=4)
                    nc.sync.dma_start(out=X0t[:],
                                      in_=xb[:, 0, :, b0c:b0c + BCH, :])
                X0f = X0t.rearrange("p k b j -> p (k b j)")
                X1d = X1t[:, 1:FCH + 1]

                scp = pool.tile([128, FCH], bf16, tag="scp", bufs=4)
                rcp = pool.tile([128, FCH], bf16, tag="rcp", bufs=4)
                u4b = pool.tile([128, FCH], bf16, tag="u4b", bufs=4)
                # DVE intermediates packed in one tile, regions 4KB apart
                # (distinct SBUF subbanks for any operand pair)
                SPR = pool.tile([128, 6, FCH], bf16, tag="spread", bufs=4)
                C1p, C1a = SPR[:, 0, :], SPR[:, 1, :]
                t3b, t2b, v3 = SPR[:, 2, :], SPR[:, 3, :], SPR[:, 4, :]
                res = SPR[:, 5, :].rearrange("p (k b j) -> p k b j",
                                             k=2, b=BCH)

                for m in range(2):
                    R1p = pp.tile([128, 1024], f32, name=f"r1_{c}_{m}",
                                  tag="r1")
                    S1p = pp.tile([128, 1024], f32, name=f"s1_{c}_{m}",
                                  tag="s1")
                    for bp in range(2):
                        # U4 rotates 2-deep so PE never waits on the stt
                        U4p = pp.tile([128, 512], f32,
                                      name=f"u4_{c}_{m}_{bp}", tag="u4",
                                      bufs=2)
                        osl = slice(bp * 512, bp * 512 + 512)
                        for kb in range(2):
                            st, sp = kb == 0, kb == 1
                            io = kb * 1024 + bp * 512
                            nc.tensor.matmul(R1p[:, osl], Wb(m * 2 + kb),
                                             X1d[:, io:io + 512],
                                             start=st, stop=sp)
                            nc.tensor.matmul(S1p[:, osl], Wb(m * 2 + kb),
                                             X0f[:, io:io + 512],
                                             start=st, stop=sp)
                            nc.tensor.matmul(U4p[:], Wb(4 + m * 2 + kb),
                                             X1d[:, io:io + 512],
                                             start=st, stop=False)
                        # col-neighbor sums: 4I @ X1 shifted +-1 (padded tile)
                        so = m * 1024 + bp * 512
                        nc.tensor.matmul(U4p[:], Wb(8),
                                         X1t[:, so + 2:so + 2 + 512],
                                         start=False, stop=False)
                        nc.tensor.matmul(U4p[:], Wb(8),
                                         X1t[:, so:so + 512],
                                         start=False, stop=True)
                        # evac U4 to bf16 (ScalarE has the slack)
                        nc.scalar.copy(out=u4b[:, so:so + 512], in_=U4p[:])
                    msl = slice(m * 1024, (m + 1) * 1024)
                    nc.scalar.copy(out=scp[:, msl], in_=S1p[:])
                    nc.scalar.copy(out=rcp[:, msl], in_=R1p[:])

                # column stencils (interior; border cols from edge pipeline)
                nc.vector.tensor_sub(C1p[:, 1:FCH - 1], X1t[:, 3:FCH + 1],
                                     X1t[:, 1:FCH - 1])
                nc.vector.tensor_sub(C1a[:, 1:FCH - 1], X0f[:, 2:FCH],
                                     X0f[:, 0:FCH - 2])
                nc.vector.tensor_mul(t3b[:], C1a[:], C1p[:])
                nc.vector.tensor_mul(v3[:], X0f[:], u4b[:])
                nc.vector.tensor_mul(t2b[:], scp[:], rcp[:])
                nc.vector.tensor_add(t2b[:], t2b[:], t3b[:])
                nc.vector.tensor_add(v3[:], v3[:], t2b[:])
                pending.append((v3, res, b0c))

            for ent in pending:
                flush(ent)

    nc.compile()
    return nc


def _get_program():
    if "nc" not in _cache:
        _cache["nc"] = _build_program()
        _cache["wtb"] = _weights_main()
        _cache["wte"] = _weights_edge()
    return _cache["nc"]


def _shard_inputs(x0_pred):
    import ml_dtypes
    x = np.asarray(x0_pred, dtype=np.float32)
    nc = _get_program()
    wtb, wte = _cache["wtb"], _cache["wte"]
    in_maps = []
    for i in range(NCORES):
        shard = x[i * BPC:(i + 1) * BPC]                      # [16,2,256,256]
        arr = shard.reshape(BPC, 2, 2, 128, P).transpose(3, 1, 2, 0, 4)
        arr = np.ascontiguousarray(arr)
        arr[:, 0] += 1.5
        arr[:, 1] *= -GAMMA
        cols = [0, 1, 2, 3, P - 4, P - 3, P - 2, P - 1]
        xe = np.ascontiguousarray(arr[:, :, :, :, cols])
        xbi = arr.astype(ml_dtypes.bfloat16)
        in_maps.append({"xb": xbi, "xe": xe, "wtbd": wtb, "wted": wte})
    return in_maps


def _unshard(results):
    outs = []
    for i in range(NCORES):
        y = results[i]["yout"].astype(np.float32)             # [128,2,16,256]
        outs.append(y.transpose(2, 1, 0, 3).reshape(BPC, 1, P, P))
    return np.ascontiguousarray(np.concatenate(outs, axis=0))


def _run(x0_pred, trace=False, tmpdir=None):
    import time
    from concourse.bass_utils import run_bass_kernel_spmd
    nc = _get_program()
    in_maps = _shard_inputs(x0_pred)
    try:
        res = run_bass_kernel_spmd(nc, in_maps, list(range(NCORES)),
                                   trace=trace, tmpdir=tmpdir)
    except Exception:
        time.sleep(2.0)
        res = run_bass_kernel_spmd(nc, in_maps, list(range(NCORES)),
                                   trace=trace, tmpdir=tmpdir)
    return _unshard(res.results), res


def kernel(x0_pred):
    out, _ = _run(x0_pred, trace=False)
    return out


# revision 31
# speedup vs baseline: 1.2178x; 1.2178x over previous
"""Trainium2 Bass kernel for nn_DarcyResidual (P=256, B=128, 8 NeuronCores).

Math (reference):
    a = (x0 + 1.5) / 0.2,  p = (x1 + 0.9) / 115
    residual = -a*(p_d00 + p_d11) - a_d0*p_d0 - a_d1*p_d1 - 1
2nd-order central differences inside, 2nd-order one-sided at borders,
h = 1/256 on both axes.

Folded form (G = 5*65536/460):
    residual = (X0+1.5)*U4' + S1*R1' + C1a*C1p' - 1
with the host pre-scaling channel 1 by -G (so every X1-linear factor
carries the -G exactly once) and pre-adding 1.5 to channel 0:
    U4' = 4*(rowD2raw + colD2raw)(X1')   R1' = rowD1raw(X1')
    S1  = rowD1raw(X0')   C1*' = colD1raw shifts (host constants cancel
    in all derivative terms; one-sided edge cols use the same scaled xe).

All-bf16 pipeline: input is a single bf16 tensor (4.2MB/core), all row
stencils are bf16 matmuls (full PE rate; odd-element rhs offsets are
legal so the col-neighbor 4I shift matmuls read the padded X1 tile
directly).  PSUM tiles are [128,1024] (2 banks) so ScalarE evacuates
each stencil with one wide ACTIVATE.  DVE does the column stencils and
the three products at 2x bf16 with unshifted frames.  Output is bf16
(host upcasts); border columns j=0,255 come from the edge pipeline via
4 tiny SWDGE DMAs that never overlap the interior stores.
"""

import numpy as np

P = 256
B = 128
NCORES = 8
BPC = B // NCORES          # images per core = 16
CHUNKS = 4
BCH = BPC // CHUNKS        # images per chunk = 4
FCH = 2 * BCH * P          # chunk free size = 2048
GAMMA = 5.0 * 65536.0 / 460.0

_cache = {}


def _mats():
    D1 = np.zeros((P, P), dtype=np.float64)
    for i in range(1, P - 1):
        D1[i, i - 1] = -1.0
        D1[i, i + 1] = 1.0
    D1[0, 0:3] = [-3.0, 4.0, -1.0]
    D1[P - 1, P - 3:P] = [1.0, -4.0, 3.0]

    D2 = np.zeros((P, P), dtype=np.float64)
    for i in range(1, P - 1):
        D2[i, i - 1] = 1.0
        D2[i, i] = -2.0
        D2[i, i + 1] = 1.0
    D2[0, 0:4] = [2.0, -5.0, 4.0, -1.0]
    D2[P - 1, P - 4:P] = [-1.0, 4.0, -5.0, 2.0]
    return D1, D2


def _weights_main():
    """bf16 [128, 9, 128]: 0-3 D1 blocks, 4-7 4*(D2-2I) blocks, 8: 4I."""
    import ml_dtypes
    D1, D2 = _mats()
    WR2 = 4.0 * (D2 - 2.0 * np.eye(P))
    wtb = np.zeros((128, 9, 128), dtype=np.float64)
    for m in range(2):
        for kb in range(2):
            blk = lambda W: W[m * 128:(m + 1) * 128, kb * 128:(kb + 1) * 128].T
            wtb[:, m * 2 + kb, :] = blk(D1)
            wtb[:, 4 + m * 2 + kb, :] = blk(WR2)
    wtb[:, 8, :] = 4.0 * np.eye(128)
    return wtb.astype(ml_dtypes.bfloat16)


def _weights_edge():
    """f32 [128, 8, 128]: 0-3 D1 blocks, 4-7 4*D2 blocks (edge pipeline)."""
    D1, D2 = _mats()
    WR2E = 4.0 * D2
    wte = np.zeros((128, 8, 128), dtype=np.float32)
    for m in range(2):
        for kb in range(2):
            blk = lambda W: W[m * 128:(m + 1) * 128, kb * 128:(kb + 1) * 128].T
            wte[:, m * 2 + kb, :] = blk(D1)
            wte[:, 4 + m * 2 + kb, :] = blk(WR2E)
    return wte


def _build_program():
    from concourse import bacc
    import concourse.mybir as mybir
    from concourse.tile import TileContext

    f32 = mybir.dt.float32
    f32r = mybir.dt.float32r
    bf16 = mybir.dt.bfloat16
    ADD = mybir.AluOpType.add
    SUB = mybir.AluOpType.subtract
    MUL = mybir.AluOpType.mult
    COPY = mybir.ActivationFunctionType.Copy

    nc = bacc.Bacc("TRN2", target_bir_lowering=False, debug=False,
                   num_devices=NCORES)
    xb = nc.dram_tensor("xb", [128, 2, 2, BPC, P], bf16, kind="ExternalInput")
    xe = nc.dram_tensor("xe", [128, 2, 2, BPC, 8], f32r, kind="ExternalInput")
    wtbd = nc.dram_tensor("wtbd", [128, 9, 128], bf16, kind="ExternalInput")
    wted = nc.dram_tensor("wted", [128, 8, 128], f32r, kind="ExternalInput")
    yout = nc.dram_tensor("yout", [128, 2, BPC, P], bf16, kind="ExternalOutput")

    with TileContext(nc) as tc:
        with (
            tc.tile_pool(name="const", bufs=1) as cpool,
            tc.tile_pool(name="edge", bufs=1) as epool,
            tc.tile_pool(name="work", bufs=2) as pool,
            tc.tile_pool(name="psum", bufs=1, space="PSUM") as pp,
        ):
            # ---- chunk-0 input first (ch1 gates the first matmuls);
            # split so the first image-pair lands early ----
            X1t0 = pool.tile([128, FCH + 2], bf16, tag="x1", bufs=4)
            nc.sync.dma_start(
                out=X1t0[:, 1:FCH + 1].rearrange(
                    "p (k b j) -> p k b j", k=2, b=BCH)[:, :, 0:2, :],
                in_=xb[:, 1, :, 0:2, :])
            wtb = cpool.tile([128, 9, 128], bf16)
            nc.sync.dma_start(out=wtb[:], in_=wtbd[:])
            wte = cpool.tile([128, 8, 128], f32r)
            nc.sync.dma_start(out=wte[:], in_=wted[:])
            X0e = epool.tile([128, 2, BPC, 8], f32r)
            X1e = epool.tile([128, 2, BPC, 8], f32r)
            nc.sync.dma_start(out=X1e[:], in_=xe[:, 1])
            nc.sync.dma_start(out=X0e[:], in_=xe[:, 0])
            X0t0 = pool.tile([128, 2, BCH, P], bf16, tag="x0", bufs=4)
            nc.sync.dma_start(out=X0t0[:, :, 0:2, :],
                              in_=xb[:, 0, :, 0:2, :])
            nc.sync.dma_start(
                out=X1t0[:, 1:FCH + 1].rearrange(
                    "p (k b j) -> p k b j", k=2, b=BCH)[:, :, 2:BCH, :],
                in_=xb[:, 1, :, 2:BCH, :])
            nc.sync.dma_start(out=X0t0[:, :, 2:BCH, :],
                              in_=xb[:, 0, :, 2:BCH, :])

            def Wb(i):
                return wtb[:, i, :]

            def We(i):
                return wte[:, i, :]

            stt = nc.vector.scalar_tensor_tensor

            # ------------- edge pipeline (output cols j=0 and j=255) -------
            X0ef = X0e.rearrange("p k b c -> p (k b c)")
            X1ef = X1e.rearrange("p k b c -> p (k b c)")
            E1 = X1e.bitcast(f32).rearrange("p k b c -> p (k b) c")
            E0 = X0e.bitcast(f32).rearrange("p k b c -> p (k b) c")

            def et(name, d=2):
                return epool.tile([128, 2 * BPC, d], f32, name=name, tag=name)

            # edge psum, 2 banks: R2e in bank0 [0:256), R1e in bank1
            # [512:768); S1e reuses bank0 AFTER U4e consumes R2e (groups in
            # a bank must be sequential, never interleaved)
            pe_t = pp.tile([128, 1024], f32, name="edgep", tag="edgep")
            R2ef = pe_t[:, 0:256]
            R1ef = pe_t[:, 512:768]
            S1ef = pe_t[:, 0:256]
            for m in range(2):
                osl = slice(m * 128, (m + 1) * 128)
                for kb in range(2):
                    isl = slice(kb * 128, (kb + 1) * 128)
                    st, sp = kb == 0, kb == 1
                    nc.tensor.matmul(R1ef[:, osl], We(m * 2 + kb),
                                     X1ef[:, isl], start=st, stop=sp)
                    nc.tensor.matmul(R2ef[:, osl], We(4 + m * 2 + kb),
                                     X1ef[:, isl], start=st, stop=sp)

            # paired forward/mirrored one-sided diffs (half 0: j=0, half 1:
            # j=255 side)
            # SBUF-only edge elementwise runs on the (otherwise idle) Pool
            # engine; only PSUM-reading ops stay on DVE
            a1, b1, c1 = et("a1"), et("b1"), et("c1")
            a0, b0 = et("a0"), et("b0")
            nc.gpsimd.tensor_sub(a1[:], E1[:, :, 1:8:6], E1[:, :, 0:7:6])
            nc.gpsimd.tensor_sub(b1[:], E1[:, :, 2:7:4], E1[:, :, 1:6:4])
            nc.gpsimd.tensor_sub(c1[:], E1[:, :, 3:6:2], E1[:, :, 2:5:2])
            nc.gpsimd.tensor_sub(a0[:], E0[:, :, 1:8:6], E0[:, :, 0:7:6])
            nc.gpsimd.tensor_sub(b0[:], E0[:, :, 2:7:4], E0[:, :, 1:6:4])

            q, Z = et("q"), et("Z")
            C1pe, C1ae = et("C1pe"), et("C1ae")
            stt(q[:], b1[:], 3.0, c1[:], MUL, SUB)      # 3b - c
            stt(Z[:], a1[:], -2.0, q[:], MUL, ADD)      # -2a + 3b - c
            stt(C1pe[:], a1[:], 3.0, b1[:], MUL, SUB)   # 3a - b
            stt(C1ae[:], a0[:], 3.0, b0[:], MUL, SUB)

            RP2 = pe_t[:, 0:256].rearrange("p (x c) -> p x c", c=8)
            RP1 = pe_t[:, 512:768].rearrange("p (x c) -> p x c", c=8)
            U4e, tme, t2e = et("U4e"), et("tme"), et("t2e")
            stt(U4e[:, :, 0:1], Z[:, :, 0:1], 4.0, RP2[:, :, 0:1], MUL, ADD)
            stt(U4e[:, :, 1:2], Z[:, :, 1:2], -4.0, RP2[:, :, 7:8], MUL, ADD)

            # S1e into bank0 now that U4e has consumed R2e
            for m in range(2):
                osl = slice(m * 128, (m + 1) * 128)
                for kb in range(2):
                    isl = slice(kb * 128, (kb + 1) * 128)
                    nc.tensor.matmul(S1ef[:, osl], We(m * 2 + kb),
                                     X0ef[:, isl], start=kb == 0,
                                     stop=kb == 1)

            Scpe = epool.tile([128, 256], f32)
            nc.scalar.copy(out=Scpe[:], in_=S1ef[:])
            SP = Scpe.rearrange("p (x c) -> p x c", c=8)

            # E0 already holds x0+1.5 (host), E1 holds -G*x1
            nc.gpsimd.tensor_mul(tme[:], E0[:, :, 0:8:7], U4e[:])
            nc.vector.tensor_mul(t2e[:], SP[:, :, 0:8:7], RP1[:, :, 0:8:7])
            nc.gpsimd.tensor_add(tme[:], tme[:], t2e[:])
            nc.gpsimd.tensor_mul(C1ae[:], C1ae[:], C1pe[:])
            nc.gpsimd.tensor_add(tme[:], tme[:], C1ae[:])
            rese = epool.tile([128, 2, BPC, 2], bf16)
            nc.scalar.activation(
                rese.rearrange("p k b e -> p (k b) e"), tme[:], COPY,
                bias=-1.0, scale=1.0)
            # border columns: 4 tiny strided DMAs, disjoint from interior
            for k in range(2):
                nc.gpsimd.dma_start(out=yout[:, k, :, 0:1],
                                    in_=rese[:, k, :, 0:1])
                nc.gpsimd.dma_start(out=yout[:, k, :, P - 1:P],
                                    in_=rese[:, k, :, 1:2])

            # ------------- main pipeline, 4 chunks of 4 images -------------
            # res finalization + store lag 2 chunks behind so the Scalar
            # res-ACT never head-of-line blocks younger chunks' evacs
            pending = []

            def flush(ent):
                v3p, resp, b0p = ent
                nc.scalar.activation(
                    resp.rearrange("p k b j -> p (k b j)"), v3p[:],
                    COPY, bias=-1.0, scale=1.0)
                for k in range(2):
                    nc.gpsimd.dma_start(
                        out=yout[:, k, b0p:b0p + BCH, 1:P - 1],
                        in_=resp[:, k, :, 1:P - 1])

            for c in range(CHUNKS):
                b0c = c * BCH
                if len(pending) >= 1:
                    flush(pending.pop(0))
                if c == 0:
                    X1t, X0t = X1t0, X0t0
                else:
                    X1t = pool.tile([128, FCH + 2], bf16, tag="x1", bufs=4)
                    nc.sync.dma_start(
                        out=X1t[:, 1:FCH + 1].rearrange(
                            "p (k b j) -> p k b j", k=2, b=BCH),
                        in_=xb[:, 1, :, b0c:b0c + BCH, :])
                    X0t = pool.tile([128, 2, BCH, P], bf16, tag="x0", bufs=4)
                    nc.sync.dma_start(out=X0t[:],
                                      in_=xb[:, 0, :, b0c:b0c + BCH, :])
                X0f = X0t.rearrange("p k b j -> p (k b j)")
                X1d = X1t[:, 1:FCH + 1]

                scp = pool.tile([128, FCH], bf16, tag="scp", bufs=4)
                rcp = pool.tile([128, FCH], bf16, tag="rcp", bufs=4)
                # DVE intermediates packed in one tile, regions 4KB apart
                # (distinct SBUF subbanks for any operand pair)
                SPR = pool.tile([128, 6, FCH], bf16, tag="spread", bufs=4)
                C1p, C1a = SPR[:, 0, :], SPR[:, 1, :]
                t3b, t2b, v3 = SPR[:, 2, :], SPR[:, 3, :], SPR[:, 4, :]
                res = SPR[:, 5, :].rearrange("p (k b j) -> p k b j",
                                             k=2, b=BCH)

                for m in range(2):
                    R1p = pp.tile([128, 1024], f32, name=f"r1_{c}_{m}",
                                  tag="r1")
                    S1p = pp.tile([128, 1024], f32, name=f"s1_{c}_{m}",
                                  tag="s1")
                    for bp in range(2):
                        # U4 rotates 2-deep so PE never waits on the stt
                        U4p = pp.tile([128, 512], f32,
                                      name=f"u4_{c}_{m}_{bp}", tag="u4",
                                      bufs=2)
                        osl = slice(bp * 512, bp * 512 + 512)
                        for kb in range(2):
                            st, sp = kb == 0, kb == 1
                            io = kb * 1024 + bp * 512
                            nc.tensor.matmul(R1p[:, osl], Wb(m * 2 + kb),
                                             X1d[:, io:io + 512],
                                             start=st, stop=sp)
                            nc.tensor.matmul(S1p[:, osl], Wb(m * 2 + kb),
                                             X0f[:, io:io + 512],
                                             start=st, stop=sp)
                            nc.tensor.matmul(U4p[:], Wb(4 + m * 2 + kb),
                                             X1d[:, io:io + 512],
                                             start=st, stop=False)
                        # col-neighbor sums: 4I @ X1 shifted +-1 (padded tile)
                        so = m * 1024 + bp * 512
                        nc.tensor.matmul(U4p[:], Wb(8),
                                         X1t[:, so + 2:so + 2 + 512],
                                         start=False, stop=False)
                        nc.tensor.matmul(U4p[:], Wb(8),
                                         X1t[:, so:so + 512],
                                         start=False, stop=True)
                        # v3 = X0' * U4 straight off PSUM (no evac)
                        qsl = slice(so, so + 512)
                        stt(v3[:, qsl], X0f[:, qsl], 1.0, U4p[:], MUL, MUL)
                    msl = slice(m * 1024, (m + 1) * 1024)
                    nc.scalar.copy(out=scp[:, msl], in_=S1p[:])
                    nc.scalar.copy(out=rcp[:, msl], in_=R1p[:])

                # column stencils (interior; border cols from edge pipeline)
                nc.vector.tensor_sub(C1p[:, 1:FCH - 1], X1t[:, 3:FCH + 1],
                                     X1t[:, 1:FCH - 1])
                nc.vector.tensor_sub(C1a[:, 1:FCH - 1], X0f[:, 2:FCH],
                                     X0f[:, 0:FCH - 2])
                nc.vector.tensor_mul(t3b[:], C1a[:], C1p[:])
                nc.vector.tensor_mul(t2b[:], scp[:], rcp[:])
                nc.vector.tensor_add(t2b[:], t2b[:], t3b[:])
                nc.vector.tensor_add(v3[:], v3[:], t2b[:])
                pending.append((v3, res, b0c))

            for ent in pending:
                flush(ent)

    nc.compile()
    return nc


def _get_program():
    if "nc" not in _cache:
        _cache["nc"] = _build_program()
        _cache["wtb"] = _weights_main()
        _cache["wte"] = _weights_edge()
    return _cache["nc"]


def _shard_inputs(x0_pred):
    import ml_dtypes
    x = np.asarray(x0_pred, dtype=np.float32)
    nc = _get_program()
    wtb, wte = _cache["wtb"], _cache["wte"]
    in_maps = []
    for i in range(NCORES):
        shard = x[i * BPC:(i + 1) * BPC]                      # [16,2,256,256]
        arr = shard.reshape(BPC, 2, 2, 128, P).transpose(3, 1, 2, 0, 4)
        arr = np.ascontiguousarray(arr)
        arr[:, 0] += 1.5
        arr[:, 1] *= -GAMMA
        cols = [0, 1, 2, 3, P - 4, P - 3, P - 2, P - 1]
        xe = np.ascontiguousarray(arr[:, :, :, :, cols])
        xbi = arr.astype(ml_dtypes.bfloat16)
        in_maps.append({"xb": xbi, "xe": xe, "wtbd": wtb, "wted": wte})
    return in_maps


def _unshard(results):
    outs = []
    for i in range(NCORES):
        y = results[i]["yout"].astype(np.float32)             # [128,2,16,256]
        outs.append(y.transpose(2, 1, 0, 3).reshape(BPC, 1, P, P))
    return np.ascontiguousarray(np.concatenate(outs, axis=0))


def _run(x0_pred, trace=False, tmpdir=None):
    import time
    from concourse.bass_utils import run_bass_kernel_spmd
    nc = _get_program()
    in_maps = _shard_inputs(x0_pred)
    try:
        res = run_bass_kernel_spmd(nc, in_maps, list(range(NCORES)),
                                   trace=trace, tmpdir=tmpdir)
    except Exception:
        time.sleep(2.0)
        res = run_bass_kernel_spmd(nc, in_maps, list(range(NCORES)),
                                   trace=trace, tmpdir=tmpdir)
    return _unshard(res.results), res


def kernel(x0_pred):
    out, _ = _run(x0_pred, trace=False)
    return out
